# revision 1
# baseline (speedup 1.0000x reference)
"""Single-head attention kernel for Trainium2, SPMD over 8 NeuronCores.

Problem: x [4,4096,1024] f32 -> q/k/v = x@W+b (head 128) -> softmax(q k^T/sqrt(128)) @ v.
Sharding: core i handles batch i//2, query half i%2. Each core receives its
batch's full x with rows rotated so its 2048 queries are rows 0:2048 (key
order is irrelevant to softmax sums), so all cores run one identical program.

Perf notes (from NTFF traces on this hardware):
- fp32 matmul runs in LOW_HIGH 2-pass mode = 4 cycles/row; fp16 is 1 cyc/row
  with an 11-bit mantissa. All values here are O(10), so the whole compute
  path runs in fp16 with fp32 PSUM accumulation (measured ~4e-4 end-to-end).
- DMA-xbar transposes interleaved with regular DMAs thrash xbar_mode and
  serialize the DMA system; transposes run on the PE in transpose-mode
  (1 cyc/row for fp16) instead.
- PSUM accumulation groups: start=True clears the WHOLE bank, so each of the
  8 P@V accumulators gets its own bank-group; P is materialized in SBUF per
  query block and consumed qs-outer so only 4 accumulator banks are live.
- exp on ScalarE costs ~(N+352)/1.2ns per instruction; issued on [128,1024]
  PSUM spans to amortize. x f32->f16 downcasts also run on ScalarE (idle in
  phase 1); PSUM->SBUF copies run on VectorE.
- P@V appends a ones-column to V so the softmax denominator lands in PSUM
  column 128 of each accumulator for free.
"""

import sys

if "/opt/trn_rl_repo" not in sys.path:
    sys.path.insert(0, "/opt/trn_rl_repo")

import numpy as np

P = 128          # partitions
S = 4096         # sequence length
E = 1024         # n_embd
D = 128          # head size
SQ = 2048        # queries per core
SC = 512         # s-processing chunk (phase 1)
NSC = S // SC    # 8
NEC = E // P     # 8
NKT = S // P     # 32 key tiles
QBLK = 1024      # phase-2 query block (ACT instruction width)
NQB = SQ // QBLK # 2
SCALE = 1.0 / float(np.sqrt(D))

_CACHE = {}


def _build_nc():
    import concourse.mybir as mybir
    import concourse.tile as tile
    from concourse import bacc

    f32 = mybir.dt.float32
    f16 = mybir.dt.float16
    AF = mybir.ActivationFunctionType

    nc = bacc.Bacc(None, target_bir_lowering=False)
    x = nc.dram_tensor("x16", [S, E], f16, kind="ExternalInput")
    wq = nc.dram_tensor("wq", [E, D], f32, kind="ExternalInput")
    wk = nc.dram_tensor("wk", [E, D], f32, kind="ExternalInput")
    wv = nc.dram_tensor("wv", [E, D], f32, kind="ExternalInput")
    bq = nc.dram_tensor("bq", [D, 1], f32, kind="ExternalInput")
    bk = nc.dram_tensor("bk", [D, 1], f32, kind="ExternalInput")
    bv = nc.dram_tensor("bv", [D, 1], f32, kind="ExternalInput")
    ident = nc.dram_tensor("ident", [P, P], f32, kind="ExternalInput")
    out = nc.dram_tensor("out", [SQ, D], f32, kind="ExternalOutput")

    with tile.TileContext(nc) as tc:
        with tc.tile_pool(name="big", bufs=1) as bigp, \
             tc.tile_pool(name="op", bufs=4) as op, \
             tc.tile_pool(name="ppe", bufs=33) as ppe:

            phase1_pools = [
                tc.tile_pool(name="const", bufs=1),
                tc.tile_pool(name="xfp", bufs=10),
                tc.tile_pool(name="xtp", bufs=10),
                tc.tile_pool(name="vtmp", bufs=2),
            ]
            constp, xfp, xtp, vtmpp = [pl.__enter__() for pl in phase1_pools]
            # --- constants in SBUF ---
            # identity first: the very first PE transposes depend on it, and
            # DMAs issue in program order on the Sync queue.
            id_st = constp.tile([P, P], f32)
            nc.sync.dma_start(out=id_st, in_=ident[:, :])
            id16 = constp.tile([P, P], f16)
            nc.vector.tensor_copy(id16, id_st)
            # prefetch the first two s-chunks of x ahead of the weight DMAs
            pre_x = []
            for pi in range(8):
                x16 = xfp.tile([P, E], f16, tag="x16", name="x16")
                nc.sync.dma_start(out=x16, in_=x[pi * P:(pi + 1) * P, :])
                pre_x.append(x16)
            w16 = []
            for nm, w_dram in (("wq", wq), ("wk", wk), ("wv", wv)):
                w_st = constp.tile([P, E], f32, name=f"{nm}_st")
                for ec in range(NEC):
                    nc.sync.dma_start(out=w_st[:, ec * P:(ec + 1) * P],
                                      in_=w_dram[ec * P:(ec + 1) * P, :])
                w_sb = constp.tile([P, E], f16, name=f"{nm}16")
                nc.vector.tensor_copy(w_sb, w_st)
                w16.append(w_sb)
            wq_sb, wk_sb, wv_sb = w16
            bq_sb = constp.tile([P, 1], f32)
            bk_sb = constp.tile([P, 1], f32)
            bv_sb = constp.tile([P, 1], f32)
            nc.sync.dma_start(out=bq_sb, in_=bq[:, :])
            nc.sync.dma_start(out=bk_sb, in_=bk[:, :])
            nc.sync.dma_start(out=bv_sb, in_=bv[:, :])

            # persistent activations (all fp16)
            kT_sb = bigp.tile([P, S], f16)        # K^T  [d, s]
            qT_sb = bigp.tile([P, SQ], f16)       # Q^T  [d, q]
            v_all = bigp.tile([P, NKT, D + 1], f16)  # [k_local, kt, 128 V | ones]
            nc.vector.memset(v_all[:, :, D:D + 1], 1.0)

            # ---------------- phase 1: x load/downcast/transpose + QKV ----------------
            p0a = []

            def s_exp(sp_pool, p_pool, qb, kt, w=QBLK, qoff=0):
                sp = sp_pool.tile([P, w], f32, tag="sp", name="sp")
                for h in range(w // SC):
                    nc.tensor.matmul(sp[:, h * SC:(h + 1) * SC],
                                     kT_sb[:, kt * P:(kt + 1) * P],
                                     qT_sb[:, qb * QBLK + qoff + h * SC:
                                           qb * QBLK + qoff + (h + 1) * SC],
                                     start=True, stop=True)
                p_sb = p_pool.tile([P, w], f16, tag="p", name="p")
                nc.scalar.activation(p_sb, sp, AF.Exp, scale=SCALE)
                return p_sb

            with tc.tile_pool(name="tp_ps", bufs=2, space="PSUM") as tp_ps, \
                 tc.tile_pool(name="proj_ps", bufs=1, space="PSUM") as proj_ps, \
                 tc.tile_pool(name="vt_ps", bufs=1, space="PSUM") as vt_ps, \
                 tc.tile_pool(name="sp1_ps", bufs=2, space="PSUM") as sp1_ps:
                for sc in range(NSC):
                    x16s = []
                    for i in range(4):
                        if sc * 4 + i < 8:
                            x16 = pre_x[sc * 4 + i]
                        else:
                            x16 = xfp.tile([P, E], f16, tag="x16", name="x16")
                            nc.sync.dma_start(
                                out=x16, in_=x[sc * SC + i * P: sc * SC + (i + 1) * P, :])
                        x16s.append(x16)
                    xTs = []
                    for ec in range(NEC):
                        tp = tp_ps.tile([P, SC], f16, tag="tp", name="tp")
                        for i in range(4):
                            nc.tensor.transpose(tp[:, i * P:(i + 1) * P],
                                                x16s[i][:, ec * P:(ec + 1) * P],
                                                id16)
                        xT = xtp.tile([P, SC], f16, tag="xT", name="xT")
                        nc.vector.tensor_copy(xT, tp)
                        xTs.append(xT)
                    pk = proj_ps.tile([P, SC], f32, tag="pk", name="pk")
                    pv = proj_ps.tile([P, SC], f32, tag="pv", name="pv")
                    pq = proj_ps.tile([P, SC], f32, tag="pq", name="pq") if sc < NSC // 2 else None
                    for ec in range(NEC):
                        st, sp_ = (ec == 0), (ec == NEC - 1)
                        nc.tensor.matmul(pk, wk_sb[:, ec * P:(ec + 1) * P], xTs[ec],
                                         start=st, stop=sp_)
                        nc.tensor.matmul(pv, wv_sb[:, ec * P:(ec + 1) * P], xTs[ec],
                                         start=st, stop=sp_)
                        if pq is not None:
                            nc.tensor.matmul(pq, wq_sb[:, ec * P:(ec + 1) * P], xTs[ec],
                                             start=st, stop=sp_)
                    if sc >= NSC // 2:
                        for t in range(2):
                            kt0 = (sc - NSC // 2) * 4 + t
                            for h in range(2):
                                p0a.append(s_exp(sp1_ps, ppe, 0, kt0, w=SC, qoff=h * SC))
                    nc.vector.tensor_scalar_add(kT_sb[:, sc * SC:(sc + 1) * SC], pk, bk_sb)
                    if pq is not None:
                        nc.vector.tensor_scalar_add(qT_sb[:, sc * SC:(sc + 1) * SC], pq, bq_sb)
                    # V: bias add (f32 psum -> f16), PE transpose, pack into v_all
                    vtmp = vtmpp.tile([P, SC], f16, tag="vtmp", name="vtmp")
                    nc.vector.tensor_scalar_add(vtmp, pv, bv_sb)
                    vt = vt_ps.tile([P, SC], f16, tag="vt", name="vt")
                    for i in range(4):
                        nc.tensor.transpose(vt[:, i * P:(i + 1) * P],
                                            vtmp[:, i * P:(i + 1) * P],
                                            id16)
                    nc.vector.tensor_copy(
                        v_all[:, sc * 4:(sc + 1) * 4, 0:D],
                        vt[:, :].rearrange("p (b c) -> p b c", c=P))
                    if sc >= NSC // 2:
                        for t in range(2, 4):
                            kt0 = (sc - NSC // 2) * 4 + t
                            for h in range(2):
                                p0a.append(s_exp(sp1_ps, ppe, 0, kt0, w=SC, qoff=h * SC))

            # phase-1-only SBUF pools released: phase 2 needs the space for
            # 64 materialized P tiles (full cross-block overlap of S/exp and P@V)
            for pl in reversed(phase1_pools):
                pl.__exit__(None, None, None)

            # ---------------- phase 2: attention ----------------
            with tc.tile_pool(name="pp", bufs=50) as pp, \
                 tc.tile_pool(name="sp_ps", bufs=2, space="PSUM") as sp_ps, \
                 tc.tile_pool(name="acc_ps", bufs=4, space="PSUM") as acc_ps:
                p_tiles = {}
                for kt in range(NKT // 2, NKT):
                    p_tiles[(0, kt)] = s_exp(sp_ps, pp, 0, kt)
                for kt in range(NKT):
                    p_tiles[(1, kt)] = s_exp(sp_ps, pp, 1, kt)

                def plhs(qb, kt, qs):
                    if qb == 0 and kt < NKT // 2:
                        return p0a[2 * kt + qs // 4][:, (qs % 4) * P:(qs % 4 + 1) * P]
                    return p_tiles[(qb, kt)][:, qs * P:(qs + 1) * P]

                for qb in range(NQB):
                    for qs in range(QBLK // P):
                        acc = acc_ps.tile([P, D + 1], f32, tag="acc", name="acc")
                        for kt in range(NKT):
                            nc.tensor.matmul(acc, plhs(qb, kt, qs),
                                             v_all[:, kt, :],
                                             start=(kt == 0), stop=(kt == NKT - 1))
                        rec = op.tile([P, 1], f32, tag="rec", name="rec")
                        nc.vector.reciprocal(rec, acc[:, D:D + 1])
                        o_sb = op.tile([P, D], f32, tag="o", name="o")
                        nc.vector.tensor_scalar_mul(o_sb, acc[:, 0:D], rec)
                        q0 = (qb * (QBLK // P) + qs) * P
                        nc.sync.dma_start(out=out[q0:q0 + P, :], in_=o_sb)
    nc.finalize()
    return nc


def _get_nc():
    if "nc" not in _CACHE:
        _CACHE["nc"] = _build_nc()
    return _CACHE["nc"]


def _in_maps(x, Wq, bq, Wk, bk, Wv, bv):
    x = np.asarray(x, dtype=np.float32).astype(np.float16)
    shared = {
        "wq": np.ascontiguousarray(np.asarray(Wq, np.float32)),
        "wk": np.ascontiguousarray(np.asarray(Wk, np.float32)),
        "wv": np.ascontiguousarray(np.asarray(Wv, np.float32)),
        "bq": np.ascontiguousarray(np.asarray(bq, np.float32).reshape(D, 1)),
        "bk": np.ascontiguousarray(np.asarray(bk, np.float32).reshape(D, 1)),
        "bv": np.ascontiguousarray(np.asarray(bv, np.float32).reshape(D, 1)),
        "ident": np.eye(P, dtype=np.float32),
    }
    maps = []
    for core in range(8):
        b, h = core // 2, core % 2
        xb = x[b] if h == 0 else np.concatenate([x[b, SQ:], x[b, :SQ]], axis=0)
        maps.append({"x16": np.ascontiguousarray(xb), **shared})
    return maps


def _assemble(results):
    out = np.empty((4, S, D), dtype=np.float32)
    for core in range(8):
        b, h = core // 2, core % 2
        out[b, h * SQ:(h + 1) * SQ] = results[core]["out"]
    return out


def kernel(x, Wq, bq, Wk, bk, Wv, bv):
    from concourse.bass_utils import run_bass_kernel_spmd

    nc = _get_nc()
    res = run_bass_kernel_spmd(nc, _in_maps(x, Wq, bq, Wk, bk, Wv, bv),
                               core_ids=list(range(8)))
    return _assemble(res.results)



# revision 9
# speedup vs baseline: 1.2279x; 1.2279x over previous
"""Single-head attention kernel for Trainium2, SPMD over 8 NeuronCores.

Problem: x [4,4096,1024] f32 -> q/k/v = x@W+b (head 128) -> softmax(q k^T/sqrt(128)) @ v.
Sharding: core i handles batch i//2, query half i%2. Each core receives its
batch's x PRE-TRANSPOSED and PE-packed on the host (xTp [128, 8, 4096] fp16 =
x^T split into 8 embedding chunks, queries rotated to columns 0:2048; key
order is irrelevant to softmax sums); all cores run one identical program.

Design (v3, from NTFF traces of v1 @157.6us and v2 @147.4us):
- All layout work is done on the host: x ships transposed+chunk-packed, the
  three weight matrices ship fp16 in stationary-operand layout packed into
  one [128, 3072] tensor. v1 burned ~14us of PE + ~27us of Vector doing x
  transposes on-chip; v2 burned a 28us DMA head issuing 88 small DMAs
  (~600ns/instruction on the one Sync queue). v3 issues one DMA per
  512-column x chunk (8 total) and two for the weights.
- S^T = K[k,d] @ Q^T[d,q] per key-tile (kt) as soon as that kT chunk and
  q-half exist; exp on ScalarE over [128,1024] spans (64 x ~1.15us,
  pipelined under the PE's ~92us of matmul streaming), kt-ascending.
- P@V keeps P stationary / V+ones moving (softmax denominator lands free in
  PSUM col 128). Sweep 1 (4 accumulator banks, kt-major) is interleaved
  with the final s_exp pairs so the PE queue never head-of-line blocks on
  an exp-gated S matmul; sweeps 2-3 (6 banks) follow, the last one
  acc-serial with inline normalize+DMA so the output trail is ~1us.
- fp32 matmul is 4 cyc/row vs fp16 1 cyc/row; whole compute path is fp16
  with fp32 PSUM accumulation (measured ~5e-4 end-to-end).
"""

import sys

if "/opt/trn_rl_repo" not in sys.path:
    sys.path.insert(0, "/opt/trn_rl_repo")

import numpy as np

P = 128          # partitions
S = 4096         # sequence length
E = 1024         # n_embd
D = 128          # head size
SQ = 2048        # queries per core
SC = 512         # s-processing chunk (phase 1)
NSC = S // SC    # 8
NEC = E // P     # 8
NKT = S // P     # 32 key tiles
QH = 1024        # exp span (half of SQ)
SCALE = 1.0 / float(np.sqrt(D))

_CACHE = {}


def _build_nc():
    import concourse.mybir as mybir
    import concourse.tile as tile
    from concourse import bacc

    f32 = mybir.dt.float32
    f16 = mybir.dt.float16
    AF = mybir.ActivationFunctionType

    nc = bacc.Bacc(None, target_bir_lowering=False)
    xTp = nc.dram_tensor("xTp16", [P, NEC, S], f16, kind="ExternalInput")
    wpk = nc.dram_tensor("wpack16", [P, 3 * E], f16, kind="ExternalInput")
    bq = nc.dram_tensor("bq", [D, 1], f32, kind="ExternalInput")
    bk = nc.dram_tensor("bk", [D, 1], f32, kind="ExternalInput")
    bv = nc.dram_tensor("bv", [D, 1], f32, kind="ExternalInput")
    ident = nc.dram_tensor("ident16", [P, P], f16, kind="ExternalInput")
    out = nc.dram_tensor("out", [SQ, D], f32, kind="ExternalOutput")

    # (kt, qh) exp emission schedule: qh0 needs qT[0:1024] (after sc1),
    # qh1 needs qT[1024:2048] (after sc3); kT tile kt lands with sc=kt//4.
    exp_sched = {sc: [] for sc in range(NSC)}
    for kt in range(16):
        exp_sched[max(kt // 4, 1)].append((kt, 0))
        exp_sched[3].append((kt, 1))
    for kt in range(16, NKT):
        exp_sched[kt // 4].append((kt, 0))
        exp_sched[kt // 4].append((kt, 1))
    for sc in exp_sched:
        exp_sched[sc].sort()

    with tile.TileContext(nc) as tc:
        with tc.tile_pool(name="const", bufs=1) as constp, \
             tc.tile_pool(name="big", bufs=1) as bigp, \
             tc.tile_pool(name="xfp", bufs=3) as xfp, \
             tc.tile_pool(name="vtmp", bufs=2) as vtmpp, \
             tc.tile_pool(name="pp", bufs=64) as pp, \
             tc.tile_pool(name="op", bufs=4) as op:

            # --- constants: DMA order = order compute needs them ---
            id16 = constp.tile([P, P], f16)
            nc.sync.dma_start(out=id16, in_=ident[:, :])
            w_sb = constp.tile([P, 3 * E], f16, name="wpack")
            nc.sync.dma_start(out=w_sb[:, 0:E], in_=wpk[:, 0:E])         # Wq
            wq_sb = w_sb[:, 0:E]
            wk_sb = w_sb[:, E:2 * E]
            wv_sb = w_sb[:, 2 * E:3 * E]

            # persistent activations (all fp16)
            kT_sb = bigp.tile([P, S], f16)        # K^T  [d, s]
            qT_sb = bigp.tile([P, SQ], f16)       # Q^T  [d, q]
            v_all = bigp.tile([P, NKT, D + 1], f16)  # [k_local, kt, 128 V | ones]

            # first x chunk before the rest of the constants: the first
            # projection only needs Wq + chunk 0
            x_tiles = []
            xt0 = xfp.tile([P, NEC, SC], f16, tag="xT", name="xT")
            nc.sync.dma_start(out=xt0, in_=xTp[:, :, 0:SC])
            x_tiles.append(xt0)
            nc.sync.dma_start(out=w_sb[:, E:3 * E], in_=wpk[:, E:3 * E])  # Wk|Wv
            bq_sb = constp.tile([P, 1], f32)
            bk_sb = constp.tile([P, 1], f32)
            bv_sb = constp.tile([P, 1], f32)
            nc.sync.dma_start(out=bq_sb, in_=bq[:, :])
            nc.sync.dma_start(out=bk_sb, in_=bk[:, :])
            nc.sync.dma_start(out=bv_sb, in_=bv[:, :])
            nc.vector.memset(v_all[:, :, D:D + 1], 1.0)

            p_tiles = {}   # (qh, kt) -> [128 k, 1024 q] f16

            def s_exp(kt, qh):
                sp = sp_ps.tile([P, QH], f32, tag="sp", name="sp")
                for h in range(QH // SC):
                    nc.tensor.matmul(sp[:, h * SC:(h + 1) * SC],
                                     kT_sb[:, kt * P:(kt + 1) * P],
                                     qT_sb[:, qh * QH + h * SC:
                                           qh * QH + (h + 1) * SC],
                                     start=True, stop=True)
                p_sb = pp.tile([P, QH], f16, tag="p", name="p")
                nc.scalar.activation(p_sb, sp, AF.Exp, scale=SCALE)
                p_tiles[(qh, kt)] = p_sb

            def pv_mm(acc, qs, kt):
                nc.tensor.matmul(
                    acc, p_tiles[(qs // 8, kt)][:, (qs % 8) * P:(qs % 8 + 1) * P],
                    v_all[:, kt, :], start=(kt == 0), stop=(kt == NKT - 1))

            def out_chain(acc, qs):
                rec = op.tile([P, 1], f32, tag="rec", name="rec")
                nc.vector.reciprocal(rec, acc[:, D:D + 1])
                o_sb = op.tile([P, D], f32, tag="o", name="o")
                nc.vector.tensor_scalar_mul(o_sb, acc[:, 0:D], rec)
                nc.sync.dma_start(out=out[qs * P:(qs + 1) * P, :], in_=o_sb)

            # ---------------- phase 1: projections + interleaved S/exp ----------------
            sp_cm = tc.tile_pool(name="sp_ps", bufs=2, space="PSUM")
            proj_cm = tc.tile_pool(name="proj_ps", bufs=1, space="PSUM")
            vt_cm = tc.tile_pool(name="vt_ps", bufs=1, space="PSUM")
            sp_ps, proj_ps, vt_ps = (sp_cm.__enter__(), proj_cm.__enter__(),
                                     vt_cm.__enter__())
            for sc in range(NSC):
                if sc > 0:
                    xt = xfp.tile([P, NEC, SC], f16, tag="xT", name="xT")
                    nc.sync.dma_start(out=xt,
                                      in_=xTp[:, :, sc * SC:(sc + 1) * SC])
                    x_tiles.append(xt)
                xt = x_tiles[sc]
                projs = []
                if sc < 4:
                    projs.append((wq_sb, qT_sb, bq_sb, "pq"))
                projs.append((wk_sb, kT_sb, bk_sb, "pk"))
                projs.append((wv_sb, None, bv_sb, "pv"))
                for w_ap, dst, b_sb, tag in projs:
                    ps = proj_ps.tile([P, SC], f32, tag=tag, name=tag)
                    for ec in range(NEC):
                        nc.tensor.matmul(ps, w_ap[:, ec * P:(ec + 1) * P],
                                         xt[:, ec, :],
                                         start=(ec == 0), stop=(ec == NEC - 1))
                    if dst is not None:
                        nc.vector.tensor_scalar_add(
                            dst[:, sc * SC:(sc + 1) * SC], ps, b_sb)
                    else:
                        # V: bias add (f32 psum -> f16), PE transpose, pack
                        vtmp = vtmpp.tile([P, SC], f16, tag="vtmp", name="vtmp")
                        nc.vector.tensor_scalar_add(vtmp, ps, b_sb)
                        vt = vt_ps.tile([P, SC], f16, tag="vt", name="vt")
                        for i in range(4):
                            nc.tensor.transpose(vt[:, i * P:(i + 1) * P],
                                                vtmp[:, i * P:(i + 1) * P],
                                                id16)
                        nc.vector.tensor_copy(
                            v_all[:, sc * 4:(sc + 1) * 4, 0:D],
                            vt[:, :].rearrange("p (b c) -> p b c", c=P))
                if sc < NSC - 1:
                    for kt, qh in exp_sched[sc]:
                        s_exp(kt, qh)
            vt_cm.__exit__(None, None, None)
            proj_cm.__exit__(None, None, None)

            # ---------------- phase 2: P@V ----------------
            # sweep 1 (qs 0..3, kt-major) interleaved with the last chunk's
            # s_exp pairs: PE stays busy on ready P@V work while ScalarE
            # drains the exp tail.
            tail = exp_sched[NSC - 1]
            with tc.tile_pool(name="acc1_ps", bufs=4, space="PSUM") as acc1:
                accs = {qs: acc1.tile([P, D + 1], f32, tag="acc", name="acc")
                        for qs in range(4)}
                for g in range(4):
                    s_exp(*tail[2 * g])
                    s_exp(*tail[2 * g + 1])
                    for kt in range(g * 8, (g + 1) * 8):
                        for qs in range(4):
                            pv_mm(accs[qs], qs, kt)
                for qs in range(4):
                    out_chain(accs[qs], qs)
            sp_cm.__exit__(None, None, None)

            with tc.tile_pool(name="acc2_ps", bufs=6, space="PSUM") as acc2:
                # sweep 2: kt-major (late exps may still be in flight)
                accs = {qs: acc2.tile([P, D + 1], f32, tag="acc", name="acc")
                        for qs in range(4, 10)}
                for kt in range(NKT):
                    for qs in range(4, 10):
                        pv_mm(accs[qs], qs, kt)
                for qs in range(4, 10):
                    out_chain(accs[qs], qs)
                # sweep 3: acc-serial with inline normalize+store so the
                # final output DMA trails the last matmul by ~1us only
                for qs in range(10, 16):
                    acc = acc2.tile([P, D + 1], f32, tag="acc", name="acc")
                    for kt in range(NKT):
                        pv_mm(acc, qs, kt)
                    out_chain(acc, qs)
    nc.finalize()
    return nc


def _get_nc():
    if "nc" not in _CACHE:
        _CACHE["nc"] = _build_nc()
    return _CACHE["nc"]


def _pack_w(w):
    # [1024, 128] -> stationary layout [128, 8*128]: chunk ec on free axis
    w16 = np.asarray(w, np.float32).astype(np.float16)
    return w16.reshape(NEC, P, D).transpose(1, 0, 2).reshape(P, E)


def _in_maps(x, Wq, bq, Wk, bk, Wv, bv):
    x = np.asarray(x, dtype=np.float32).astype(np.float16)
    shared = {
        "wpack16": np.ascontiguousarray(
            np.concatenate([_pack_w(Wq), _pack_w(Wk), _pack_w(Wv)], axis=1)),
        "bq": np.ascontiguousarray(np.asarray(bq, np.float32).reshape(D, 1)),
        "bk": np.ascontiguousarray(np.asarray(bk, np.float32).reshape(D, 1)),
        "bv": np.ascontiguousarray(np.asarray(bv, np.float32).reshape(D, 1)),
        "ident16": np.eye(P, dtype=np.float16),
    }
    maps = []
    for core in range(8):
        b, h = core // 2, core % 2
        xb = x[b] if h == 0 else np.concatenate([x[b, SQ:], x[b, :SQ]], axis=0)
        # x^T [1024, 4096] -> [128, 8, 4096]: partition = e % 128, chunk ec
        xtp = xb.T.reshape(NEC, P, S).transpose(1, 0, 2)
        maps.append({"xTp16": np.ascontiguousarray(xtp), **shared})
    return maps


def _assemble(results):
    out = np.empty((4, S, D), dtype=np.float32)
    for core in range(8):
        b, h = core // 2, core % 2
        out[b, h * SQ:(h + 1) * SQ] = results[core]["out"]
    return out


def kernel(x, Wq, bq, Wk, bk, Wv, bv):
    from concourse.bass_utils import run_bass_kernel_spmd

    nc = _get_nc()
    res = run_bass_kernel_spmd(nc, _in_maps(x, Wq, bq, Wk, bk, Wv, bv),
                               core_ids=list(range(8)))
    return _assemble(res.results)


# revision 10
# speedup vs baseline: 1.2568x; 1.0235x over previous
"""Single-head attention kernel for Trainium2, SPMD over 8 NeuronCores.

Problem: x [4,4096,1024] f32 -> q/k/v = x@W+b (head 128) -> softmax(q k^T/sqrt(128)) @ v.
Sharding: core i handles batch i//2, query half i%2. Each core receives its
batch's x PRE-TRANSPOSED and PE-packed on the host (xTp [128, 8 sc, 8 ec, 512]
fp16, queries rotated to the front; key order is irrelevant to softmax sums);
all cores run one identical program.

Design (v4; NTFF-trace driven: v1 157.6us -> v2 147.4 -> v3 128.4):
- All layout work on the host: x ships transposed + packed so each
  512-column chunk is ONE DMA of 128x8KB contiguous descriptors (v3's
  1KB-descriptor pattern ran at ~139GB/s, this runs at full ~360GB/s);
  weights ship fp16 in stationary layout packed as [128, 3072] (2 DMAs);
  the three biases pack into one [128, 3] tensor (v3 spent 2.9us of queue
  time on three 4B-element DMAs).
- Projections: W chunk stationary, x^T chunk moving, fp32 PSUM, bias-add
  on VectorE doubles as the PSUM->SBUF f16 downcast.
- S^T = K[k,d] @ Q^T[d,q] per key-tile kt, emitted as soon as its kT chunk
  and q-span exist -- kt 0..3 go in 512-wide q-halves right after chunk 0
  so ScalarE (the second-busiest engine, ~74us of exp) starts ~16us in and
  never becomes the tail. exp spans [128,1024] otherwise, kt-ascending.
- P@V keeps P stationary / V+ones moving (softmax denominator lands free
  in PSUM col 128). Sweep 1 (4 accumulator banks, kt 0..27 kt-major,
  interleaved with the last chunk's s_exps, then kt 28..31); sweeps 2-3
  (6 banks) follow, the last acc-serial with inline normalize+DMA so the
  final output DMA trails the last matmul by ~1us.
- Whole compute path fp16 (1 cyc/row vs 4 for fp32) with fp32 PSUM;
  measured ~5e-4 end-to-end rel err.
"""

import sys

if "/opt/trn_rl_repo" not in sys.path:
    sys.path.insert(0, "/opt/trn_rl_repo")

import numpy as np

P = 128          # partitions
S = 4096         # sequence length
E = 1024         # n_embd
D = 128          # head size
SQ = 2048        # queries per core
SC = 512         # s-processing chunk (phase 1)
NSC = S // SC    # 8
NEC = E // P     # 8
NKT = S // P     # 32 key tiles
QH = 1024        # exp span (half of SQ)
SCALE = 1.0 / float(np.sqrt(D))

_CACHE = {}


def _build_nc():
    import concourse.mybir as mybir
    import concourse.tile as tile
    from concourse import bacc

    f32 = mybir.dt.float32
    f16 = mybir.dt.float16
    AF = mybir.ActivationFunctionType

    nc = bacc.Bacc(None, target_bir_lowering=False)
    xTp = nc.dram_tensor("xTp16", [P, NSC, NEC, SC], f16, kind="ExternalInput")
    wpk = nc.dram_tensor("wpack16", [P, 3 * E], f16, kind="ExternalInput")
    bqkv = nc.dram_tensor("bqkv", [P, 3], f32, kind="ExternalInput")
    ident = nc.dram_tensor("ident16", [P, P], f16, kind="ExternalInput")
    out = nc.dram_tensor("out", [SQ, D], f32, kind="ExternalOutput")

    with tile.TileContext(nc) as tc:
        with tc.tile_pool(name="const", bufs=1) as constp, \
             tc.tile_pool(name="big", bufs=1) as bigp, \
             tc.tile_pool(name="xfp", bufs=3) as xfp, \
             tc.tile_pool(name="vtmp", bufs=2) as vtmpp, \
             tc.tile_pool(name="pp", bufs=64) as pp, \
             tc.tile_pool(name="op", bufs=4) as op:

            # --- DMAs in the order compute needs them; first matmul only
            # needs Wq + x chunk 0 ---
            w_sb = constp.tile([P, 3 * E], f16, name="wpack")
            nc.sync.dma_start(out=w_sb[:, 0:E], in_=wpk[:, 0:E])          # Wq
            wq_sb, wk_sb, wv_sb = (w_sb[:, 0:E], w_sb[:, E:2 * E],
                                   w_sb[:, 2 * E:3 * E])
            x_tiles = [xfp.tile([P, NEC, SC], f16, tag="xT", name="xT")]
            nc.sync.dma_start(out=x_tiles[0], in_=xTp[:, 0, :, :])
            nc.sync.dma_start(out=w_sb[:, E:3 * E], in_=wpk[:, E:3 * E])  # Wk|Wv
            id16 = constp.tile([P, P], f16)
            nc.sync.dma_start(out=id16, in_=ident[:, :])
            b_sb = constp.tile([P, 3], f32, name="bqkv")
            nc.sync.dma_start(out=b_sb, in_=bqkv[:, :])
            bq_sb, bk_sb, bv_sb = b_sb[:, 0:1], b_sb[:, 1:2], b_sb[:, 2:3]

            # persistent activations (all fp16)
            kT_sb = bigp.tile([P, S], f16)        # K^T  [d, s]
            qT_sb = bigp.tile([P, SQ], f16)       # Q^T  [d, q]
            v_all = bigp.tile([P, NKT, D + 1], f16)  # [k_local, kt, 128 V | ones]
            nc.vector.memset(v_all[:, :, D:D + 1], 1.0)

            p_tiles = {}   # (qh, kt) -> [128 k, 1024 q] f16

            def p_tile(kt, qh):
                key = (qh, kt)
                if key not in p_tiles:
                    p_tiles[key] = pp.tile([P, QH], f16, tag="p", name="p")
                return p_tiles[key]

            def s_exp(kt, qh):
                sp = sp_ps.tile([P, QH], f32, tag="sp", name="sp")
                for h in range(QH // SC):
                    nc.tensor.matmul(sp[:, h * SC:(h + 1) * SC],
                                     kT_sb[:, kt * P:(kt + 1) * P],
                                     qT_sb[:, qh * QH + h * SC:
                                           qh * QH + (h + 1) * SC],
                                     start=True, stop=True)
                nc.scalar.activation(p_tile(kt, qh), sp, AF.Exp, scale=SCALE)

            def s_exp_half2(kta, ktb, qh, h):
                # one [128,1024] PSUM tile carries the 512-wide q-half S
                # blocks of TWO kt tiles; one exp per kt-half
                sp = sp_ps.tile([P, QH], f32, tag="sp", name="sp")
                for i, kt in enumerate((kta, ktb)):
                    nc.tensor.matmul(sp[:, i * SC:(i + 1) * SC],
                                     kT_sb[:, kt * P:(kt + 1) * P],
                                     qT_sb[:, qh * QH + h * SC:
                                           qh * QH + (h + 1) * SC],
                                     start=True, stop=True)
                for i, kt in enumerate((kta, ktb)):
                    nc.scalar.activation(
                        p_tile(kt, qh)[:, h * SC:(h + 1) * SC],
                        sp[:, i * SC:(i + 1) * SC], AF.Exp, scale=SCALE)

            def pv_mm(acc, qs, kt):
                nc.tensor.matmul(
                    acc, p_tiles[(qs // 8, kt)][:, (qs % 8) * P:(qs % 8 + 1) * P],
                    v_all[:, kt, :], start=(kt == 0), stop=(kt == NKT - 1))

            def out_chain(acc, qs):
                rec = op.tile([P, 1], f32, tag="rec", name="rec")
                nc.vector.reciprocal(rec, acc[:, D:D + 1])
                o_sb = op.tile([P, D], f32, tag="o", name="o")
                nc.vector.tensor_scalar_mul(o_sb, acc[:, 0:D], rec)
                nc.sync.dma_start(out=out[qs * P:(qs + 1) * P, :], in_=o_sb)

            # exp emission schedule per chunk: (kt, qh) pairs, kt-ascending.
            # kt 0..3 x qh0 go in 512 halves (h0 after sc0, h1 after sc1) via
            # s_exp_half2; qh0 of kt>=4 after max(sc(kt),1); qh1 after
            # max(sc(kt),3).
            exp_sched = {sc: [] for sc in range(NSC)}
            for kt in range(4, 16):
                exp_sched[max(kt // 4, 1)].append((kt, 0))
                exp_sched[3].append((kt, 1))
            for kt in range(0, 4):
                exp_sched[3].append((kt, 1))
            for kt in range(16, NKT):
                exp_sched[kt // 4].append((kt, 0))
                exp_sched[kt // 4].append((kt, 1))
            for sc in exp_sched:
                exp_sched[sc].sort()

            # ---------------- phase 1: projections + interleaved S/exp ----------------
            sp_cm = tc.tile_pool(name="sp_ps", bufs=2, space="PSUM")
            proj_cm = tc.tile_pool(name="proj_ps", bufs=1, space="PSUM")
            vt_cm = tc.tile_pool(name="vt_ps", bufs=1, space="PSUM")
            sp_ps, proj_ps, vt_ps = (sp_cm.__enter__(), proj_cm.__enter__(),
                                     vt_cm.__enter__())
            for sc in range(NSC):
                if sc > 0:
                    xt = xfp.tile([P, NEC, SC], f16, tag="xT", name="xT")
                    nc.sync.dma_start(out=xt, in_=xTp[:, sc, :, :])
                    x_tiles.append(xt)
                xt = x_tiles[sc]
                projs = []
                if sc < 4:
                    projs.append((wq_sb, qT_sb, bq_sb, "pq"))
                projs.append((wk_sb, kT_sb, bk_sb, "pk"))
                projs.append((wv_sb, None, bv_sb, "pv"))
                for w_ap, dst, bias, tag in projs:
                    ps = proj_ps.tile([P, SC], f32, tag=tag, name=tag)
                    for ec in range(NEC):
                        nc.tensor.matmul(ps, w_ap[:, ec * P:(ec + 1) * P],
                                         xt[:, ec, :],
                                         start=(ec == 0), stop=(ec == NEC - 1))
                    if dst is not None:
                        nc.vector.tensor_scalar_add(
                            dst[:, sc * SC:(sc + 1) * SC], ps, bias)
                    else:
                        # V: bias add (f32 psum -> f16), PE transpose, pack
                        vtmp = vtmpp.tile([P, SC], f16, tag="vtmp", name="vtmp")
                        nc.vector.tensor_scalar_add(vtmp, ps, bias)
                        vt = vt_ps.tile([P, SC], f16, tag="vt", name="vt")
                        for i in range(4):
                            nc.tensor.transpose(vt[:, i * P:(i + 1) * P],
                                                vtmp[:, i * P:(i + 1) * P],
                                                id16)
                        nc.vector.tensor_copy(
                            v_all[:, sc * 4:(sc + 1) * 4, 0:D],
                            vt[:, :].rearrange("p (b c) -> p b c", c=P))
                if sc < 2:
                    # earliest exps: kt 0..3 x q-half (sc0: h0, sc1: h1)
                    s_exp_half2(0, 1, 0, sc)
                    s_exp_half2(2, 3, 0, sc)
                if sc < NSC - 1:
                    for kt, qh in exp_sched[sc]:
                        s_exp(kt, qh)
            vt_cm.__exit__(None, None, None)
            proj_cm.__exit__(None, None, None)

            # ---------------- phase 2: P@V ----------------
            # sweep 1 (qs 0..3): kt 0..27 kt-major, interleaved with the
            # last chunk's s_exp pairs; kt 28..31 last so their exps have
            # the whole sweep to land.
            tail = exp_sched[NSC - 1]
            with tc.tile_pool(name="acc1_ps", bufs=4, space="PSUM") as acc1:
                accs = {qs: acc1.tile([P, D + 1], f32, tag="acc", name="acc")
                        for qs in range(4)}
                for g in range(4):
                    s_exp(*tail[2 * g])
                    s_exp(*tail[2 * g + 1])
                    for kt in range(g * 7, (g + 1) * 7):
                        for qs in range(4):
                            pv_mm(accs[qs], qs, kt)
                for kt in range(28, NKT):
                    for qs in range(4):
                        pv_mm(accs[qs], qs, kt)
                for qs in range(4):
                    out_chain(accs[qs], qs)
            sp_cm.__exit__(None, None, None)

            with tc.tile_pool(name="acc2_ps", bufs=6, space="PSUM") as acc2:
                # sweep 2: kt-major (late exps may still be in flight)
                accs = {qs: acc2.tile([P, D + 1], f32, tag="acc", name="acc")
                        for qs in range(4, 10)}
                for kt in range(NKT):
                    for qs in range(4, 10):
                        pv_mm(accs[qs], qs, kt)
                for qs in range(4, 10):
                    out_chain(accs[qs], qs)
                # sweep 3: acc-serial with inline normalize+store so the
                # final output DMA trails the last matmul by ~1us only
                for qs in range(10, 16):
                    acc = acc2.tile([P, D + 1], f32, tag="acc", name="acc")
                    for kt in range(NKT):
                        pv_mm(acc, qs, kt)
                    out_chain(acc, qs)
    nc.finalize()
    return nc


def _get_nc():
    if "nc" not in _CACHE:
        _CACHE["nc"] = _build_nc()
    return _CACHE["nc"]


def _pack_w(w):
    # [1024, 128] -> stationary layout [128, 8*128]: chunk ec on free axis
    w16 = np.asarray(w, np.float32).astype(np.float16)
    return w16.reshape(NEC, P, D).transpose(1, 0, 2).reshape(P, E)


def _in_maps(x, Wq, bq, Wk, bk, Wv, bv):
    x = np.asarray(x, dtype=np.float32).astype(np.float16)
    shared = {
        "wpack16": np.ascontiguousarray(
            np.concatenate([_pack_w(Wq), _pack_w(Wk), _pack_w(Wv)], axis=1)),
        "bqkv": np.ascontiguousarray(np.stack(
            [np.asarray(b, np.float32) for b in (bq, bk, bv)], axis=1)),
        "ident16": np.eye(P, dtype=np.float16),
    }
    maps = []
    for core in range(8):
        b, h = core // 2, core % 2
        xb = x[b] if h == 0 else np.concatenate([x[b, SQ:], x[b, :SQ]], axis=0)
        # x^T [1024, 4096] -> [128(p), 8(sc), 8(ec), 512]: one 8KB-contiguous
        # read per partition per chunk DMA
        xtp = xb.T.reshape(NEC, P, NSC, SC).transpose(1, 2, 0, 3)
        maps.append({"xTp16": np.ascontiguousarray(xtp), **shared})
    return maps


def _assemble(results):
    out = np.empty((4, S, D), dtype=np.float32)
    for core in range(8):
        b, h = core // 2, core % 2
        out[b, h * SQ:(h + 1) * SQ] = results[core]["out"]
    return out


def kernel(x, Wq, bq, Wk, bk, Wv, bv):
    from concourse.bass_utils import run_bass_kernel_spmd

    nc = _get_nc()
    res = run_bass_kernel_spmd(nc, _in_maps(x, Wq, bq, Wk, bk, Wv, bv),
                               core_ids=list(range(8)))
    return _assemble(res.results)
